# revision 28
# baseline (speedup 1.0000x reference)
"""Conv2d(256->256, 3x3, pad=1) on 8 TRN2 NeuronCores.

Sharding: data-parallel over output rows (H). Each core computes all 256
output channels for a 28-row slice of the output; the kernel (weights) are
replicated. This keeps the PE array fully loaded (M=128 output channels per
matmul) vs. out-channel sharding which would leave M=32.

Per core the conv is an implicit GEMM: out[o, h, w] = sum over (c, kh, kw) of
xpad[c, h+kh, w+kw] * k[o, c, kh, kw]. Contraction = 2 c-blocks x 9 taps = 18
accumulating matmuls per PSUM tile of [128 o, 2 h-rows x 224 w = 448].
Matmuls run in float32r (fp32 data streamed at bf16 rate — 4x faster than
fp32 matmul, ~1.4e-4 L2 rel err measured on HW vs fp64 at this contraction
depth; fp32 would be ~1.6e-7 but 4 cycles/row).

Measured on trn2 (8 cores, max over cores): ~124 us HW exec. Breakdown:
~6 us framework preamble, ~7 us DMA gate (hidden behind PE warmup matmuls
that keep the HAM clock-gate at 8/8), ~105 us dense matmul stream (504 MMs,
~207 ns cadence, LDWEIGHTS-for-f32r at ~190 ns is the co-bottleneck and is
unavoidable: f32r matmuls must self-load weights), ~6 us drain/teardown.
"""

import sys

sys.path.insert(0, "/opt/trn_rl_repo")

import numpy as np

import concourse.mybir as mybir
from concourse import bacc
from concourse.tile import TileContext
from concourse.bass_utils import run_bass_kernel_spmd

N_CORES = 8
C, H, W = 256, 224, 224
O = 256
KH = KW = 3
HS = H // N_CORES          # 28 output rows per core
HROWS = 2                  # output rows per PSUM tile (N = 2*224 = 448)
CB = C // 128              # c blocks
OB = O // 128              # o blocks

_CACHE = {}
LAST_RESULTS = None        # test.py reads exec_time_ns / trace path from here
TRACE = False


def _build():
    nc = bacc.Bacc(None, target_bir_lowering=False)

    xs = nc.dram_tensor(
        "xs", [CB, 128, HS + 2, W + 2], mybir.dt.float32r, kind="ExternalInput"
    )
    w = nc.dram_tensor(
        "w", [CB, OB, 128, KH * KW, 128], mybir.dt.float32r, kind="ExternalInput"
    )
    out = nc.dram_tensor(
        "out", [OB, 128, HS, W], mybir.dt.float32, kind="ExternalOutput"
    )

    n_warm = 18
    with TileContext(nc) as tc:
        with (
            tc.tile_pool(name="warm", bufs=1) as pwarm,
            tc.tile_pool(name="win", bufs=1) as pw,
            tc.tile_pool(name="xin", bufs=1) as px,
            tc.tile_pool(name="psumw", bufs=1, space="PSUM") as ppw,
            tc.tile_pool(name="psum", bufs=7, space="PSUM") as pp,
            tc.tile_pool(name="outp", bufs=4) as po,
        ):
            # PE warmup: dummy matmuls on a memset tile while input DMAs
            # stream, so the HAM clock-gate is at 8/8 when real work starts.
            wt0 = pwarm.tile([128, 256], mybir.dt.bfloat16, tag="warm")
            ps0 = ppw.tile([128, 256], mybir.dt.float32, tag="warmps")
            nc.vector.memset(wt0[:], 0.0)
            for _ in range(n_warm):
                nc.tensor.matmul(ps0[:], wt0[:, :128], wt0[:], start=True, stop=True)

            # One big x tile per c-block, filled by 2-row chunked DMAs so the
            # first matmuls only wait on the first rows, not the whole tile.
            x_sb = [
                px.tile(
                    [128, HS + 2, W + 2], mybir.dt.float32r, tag=f"x{b}", name=f"x{b}"
                )
                for b in range(CB)
            ]
            w_sb = [
                pw.tile(
                    [128, KH * KW, O], mybir.dt.float32r, tag=f"w{b}", name=f"w{b}"
                )
                for b in range(CB)
            ]
            # Gate DMAs in exact consumption order of the c-block-split head
            # schedule below: b0 pieces (both ob weight halves) first so four
            # half-groups of b0 work can run while b1's pieces stream in.
            def dma_w(b, ob):
                nc.sync.dma_start(
                    out=w_sb[b][:, :, ob * 128 : (ob + 1) * 128], in_=w[b, ob]
                )

            def dma_x(b, r0, r1):
                nc.sync.dma_start(
                    out=x_sb[b][:, r0:r1, :], in_=xs[b, :, r0:r1, :]
                )

            # First pieces small (fast issue + arrival), later pieces big:
            # each dma_start costs ~0.7 us of Sync descriptor generation, so
            # the gate uses few instructions once the PE is streaming.
            nc.sync.dma_start(out=w_sb[0][:, 0:3, 0:128], in_=w[0, 0, :, 0:3, :])
            dma_x(0, 0, 2)
            nc.sync.dma_start(out=w_sb[0][:, 3:9, 0:128], in_=w[0, 0, :, 3:9, :])
            dma_x(0, 2, 8)
            dma_w(0, 1)
            dma_x(1, 0, 8)
            dma_w(1, 0)
            dma_w(1, 1)
            dma_x(0, 8, 12)
            dma_x(1, 8, 12)
            dma_x(0, 12, HS + 2)
            dma_x(1, 12, HS + 2)

            n_acc = CB * KH * KW

            def mm_group(ps, h0, ob, bs, first, last):
                idx = 0
                for b in bs:
                    for k in range(KH * KW):
                        kh, kw = divmod(k, KW)
                        nc.tensor.matmul(
                            ps[:],
                            w_sb[b][:, k, ob * 128 : (ob + 1) * 128],
                            x_sb[b][:, h0 + kh : h0 + kh + HROWS, kw : kw + W],
                            start=(first and idx == 0),
                            stop=(last and idx == len(bs) * KH * KW - 1),
                        )
                        idx += 1

            def finish_group(ps, h0, ob):
                ot = po.tile([128, HROWS, W], mybir.dt.float32, tag="ot", name="ot")
                nc.vector.tensor_copy(out=ot[:], in_=ps[:])
                nc.sync.dma_start(out=out[ob, :, h0 : h0 + HROWS, :], in_=ot[:])

            # First three bands: run the b=0 halves of six groups (3 bands x
            # 2 ob) while b=1's weights/rows are still in flight, then add
            # the b=1 halves in the same arrival order. Keeps the PE dense
            # from the moment the first ~0.4 MB lands.
            head = [(0, 0), (2, 0), (4, 0), (0, 1), (2, 1), (4, 1)]
            ps_head = {}
            for h0, ob in head:
                ps = pp.tile([128, HROWS, W], mybir.dt.float32, tag="ps", name="ps")
                ps_head[(h0, ob)] = ps
                mm_group(ps, h0, ob, [0], first=True, last=False)
            for h0, ob in [(0, 0), (0, 1), (2, 0), (2, 1), (4, 0), (4, 1)]:
                ps = ps_head[(h0, ob)]
                mm_group(ps, h0, ob, [1], first=False, last=True)
                finish_group(ps, h0, ob)

            for h0 in range(3 * HROWS, HS, HROWS):
                for ob in range(OB):
                    ps = pp.tile([128, HROWS, W], mybir.dt.float32, tag="ps", name="ps")
                    mm_group(ps, h0, ob, list(range(CB)), first=True, last=True)
                    finish_group(ps, h0, ob)

    nc.compile()
    return nc


def kernel(x: np.ndarray, kernel: np.ndarray) -> np.ndarray:
    global LAST_RESULTS
    if "nc" not in _CACHE:
        _CACHE["nc"] = _build()
    nc = _CACHE["nc"]

    x = np.ascontiguousarray(x, dtype=np.float32)
    kw_arr = np.ascontiguousarray(kernel, dtype=np.float32)

    xp = np.pad(x, ((0, 0), (1, 1), (1, 1)))          # [C, H+2, W+2]
    xp = xp.reshape(CB, 128, H + 2, W + 2)
    # w_t[b, ob, p, k, oc] = kernel[ob*128+oc, b*128+p, kh, kw] — each (b, ob)
    # quarter is contiguous per partition for a clean DMA line.
    w_t = np.ascontiguousarray(
        kw_arr.transpose(1, 2, 3, 0)
        .reshape(CB, 128, KH * KW, OB, 128)
        .transpose(0, 3, 1, 2, 4)
    )

    in_maps = []
    for i in range(N_CORES):
        xs_i = np.ascontiguousarray(xp[:, :, i * HS : i * HS + HS + 2, :])
        in_maps.append({"xs": xs_i, "w": w_t})

    results = run_bass_kernel_spmd(
        nc, in_maps, core_ids=list(range(N_CORES)), trace=TRACE
    )
    LAST_RESULTS = results

    parts = [r["out"].reshape(O, HS, W) for r in results.results]
    return np.concatenate(parts, axis=1)


# revision 29
# speedup vs baseline: 1.0523x; 1.0523x over previous
"""Conv2d(256->256, 3x3, pad=1) on 8 TRN2 NeuronCores.

Sharding: data-parallel over output rows (H). Each core computes all 256
output channels for a 28-row slice of the output; the kernel (weights) are
replicated. This keeps the PE array fully loaded (M=128 output channels per
matmul) vs. out-channel sharding which would leave M=32.

Per core the conv is an implicit GEMM: out[o, h, w] = sum over (c, kh, kw) of
xpad[c, h+kh, w+kw] * k[o, c, kh, kw]. Contraction = 2 c-blocks x 9 taps = 18
accumulating matmuls per PSUM tile of [128 o, 2 h-rows x 224 w = 448].
Matmuls run in float32r (fp32 data streamed at bf16 rate — 4x faster than
fp32 matmul, ~1.4e-4 L2 rel err measured on HW vs fp64 at this contraction
depth; fp32 would be ~1.6e-7 but 4 cycles/row).

Measured on trn2 (8 cores, max over cores): ~124 us HW exec. Breakdown:
~6 us framework preamble, ~7 us DMA gate (hidden behind PE warmup matmuls
that keep the HAM clock-gate at 8/8), ~105 us dense matmul stream (504 MMs,
~207 ns cadence, LDWEIGHTS-for-f32r at ~190 ns is the co-bottleneck and is
unavoidable: f32r matmuls must self-load weights), ~6 us drain/teardown.
"""

import sys

sys.path.insert(0, "/opt/trn_rl_repo")

import numpy as np

import concourse.mybir as mybir
from concourse import bacc
from concourse.tile import TileContext
from concourse.bass_utils import run_bass_kernel_spmd

N_CORES = 8
C, H, W = 256, 224, 224
O = 256
KH = KW = 3
HS = H // N_CORES          # 28 output rows per core
HROWS = 2                  # output rows per PSUM tile (N = 2*224 = 448)
CB = C // 128              # c blocks
OB = O // 128              # o blocks

_CACHE = {}
LAST_RESULTS = None        # test.py reads exec_time_ns / trace path from here
TRACE = False


def _build():
    nc = bacc.Bacc(None, target_bir_lowering=False)

    xs = nc.dram_tensor(
        "xs", [CB, 128, HS + 2, W + 2], mybir.dt.float32r, kind="ExternalInput"
    )
    w = nc.dram_tensor(
        "w", [CB, OB, 128, KH * KW, 128], mybir.dt.float32r, kind="ExternalInput"
    )
    out = nc.dram_tensor(
        "out", [OB, 128, HS, W], mybir.dt.float32, kind="ExternalOutput"
    )

    n_warm = 18
    with TileContext(nc) as tc:
        with (
            tc.tile_pool(name="warm", bufs=1) as pwarm,
            tc.tile_pool(name="win", bufs=1) as pw,
            tc.tile_pool(name="xin", bufs=1) as px,
            tc.tile_pool(name="psumw", bufs=1, space="PSUM") as ppw,
            tc.tile_pool(name="psum", bufs=7, space="PSUM") as pp,
            tc.tile_pool(name="outp", bufs=4) as po,
        ):
            # PE warmup: dummy matmuls on a memset tile while input DMAs
            # stream, so the HAM clock-gate is at 8/8 when real work starts.
            wt0 = pwarm.tile([128, 256], mybir.dt.bfloat16, tag="warm")
            ps0 = ppw.tile([128, 256], mybir.dt.float32, tag="warmps")
            nc.vector.memset(wt0[:], 0.0)
            for _ in range(n_warm):
                nc.tensor.matmul(ps0[:], wt0[:, :128], wt0[:], start=True, stop=True)

            # One big x tile per c-block, filled by 2-row chunked DMAs so the
            # first matmuls only wait on the first rows, not the whole tile.
            x_sb = [
                px.tile(
                    [128, HS + 2, W + 2], mybir.dt.float32r, tag=f"x{b}", name=f"x{b}"
                )
                for b in range(CB)
            ]
            w_sb = [
                pw.tile(
                    [128, KH * KW, O], mybir.dt.float32r, tag=f"w{b}", name=f"w{b}"
                )
                for b in range(CB)
            ]
            # Gate DMAs in exact consumption order of the c-block-split head
            # schedule below: b0 pieces (both ob weight halves) first so four
            # half-groups of b0 work can run while b1's pieces stream in.
            def dma_w(b, ob):
                nc.sync.dma_start(
                    out=w_sb[b][:, :, ob * 128 : (ob + 1) * 128], in_=w[b, ob]
                )

            def dma_x(b, r0, r1):
                nc.sync.dma_start(
                    out=x_sb[b][:, r0:r1, :], in_=xs[b, :, r0:r1, :]
                )

            # First weight quarter split at tap granularity so the very first
            # matmuls gate on ~0.4 MB instead of ~1 MB; x rows in 2-row pieces
            # ordered to match the head schedule's consumption order.
            nc.sync.dma_start(out=w_sb[0][:, 0:3, 0:128], in_=w[0, 0, :, 0:3, :])
            dma_x(0, 0, 2)
            dma_x(0, 2, 4)
            nc.sync.dma_start(out=w_sb[0][:, 3:9, 0:128], in_=w[0, 0, :, 3:9, :])
            dma_x(0, 4, 6)
            dma_x(0, 6, 8)
            dma_w(0, 1)
            dma_x(1, 0, 2)
            dma_x(1, 2, 4)
            dma_x(1, 4, 6)
            dma_w(1, 0)
            dma_x(1, 6, 8)
            dma_w(1, 1)
            for r in range(8, HS + 2, 2):
                for b in range(CB):
                    dma_x(b, r, r + 2)

            n_acc = CB * KH * KW

            def mm_group(ps, h0, ob, bs, first, last):
                idx = 0
                for b in bs:
                    for k in range(KH * KW):
                        kh, kw = divmod(k, KW)
                        nc.tensor.matmul(
                            ps[:],
                            w_sb[b][:, k, ob * 128 : (ob + 1) * 128],
                            x_sb[b][:, h0 + kh : h0 + kh + HROWS, kw : kw + W],
                            start=(first and idx == 0),
                            stop=(last and idx == len(bs) * KH * KW - 1),
                        )
                        idx += 1

            def finish_group(ps, h0, ob):
                ot = po.tile([128, HROWS, W], mybir.dt.float32, tag="ot", name="ot")
                nc.vector.tensor_copy(out=ot[:], in_=ps[:])
                nc.sync.dma_start(out=out[ob, :, h0 : h0 + HROWS, :], in_=ot[:])

            # First three bands: run the b=0 halves of six groups (3 bands x
            # 2 ob) while b=1's weights/rows are still in flight, then add
            # the b=1 halves in the same arrival order. Keeps the PE dense
            # from the moment the first ~0.4 MB lands.
            head = [(0, 0), (2, 0), (4, 0), (0, 1), (2, 1), (4, 1)]
            ps_head = {}
            for h0, ob in head:
                ps = pp.tile([128, HROWS, W], mybir.dt.float32, tag="ps", name="ps")
                ps_head[(h0, ob)] = ps
                mm_group(ps, h0, ob, [0], first=True, last=False)
            for h0, ob in [(0, 0), (0, 1), (2, 0), (2, 1), (4, 0), (4, 1)]:
                ps = ps_head[(h0, ob)]
                mm_group(ps, h0, ob, [1], first=False, last=True)
                finish_group(ps, h0, ob)

            for h0 in range(3 * HROWS, HS, HROWS):
                for ob in range(OB):
                    ps = pp.tile([128, HROWS, W], mybir.dt.float32, tag="ps", name="ps")
                    mm_group(ps, h0, ob, list(range(CB)), first=True, last=True)
                    finish_group(ps, h0, ob)

    nc.compile()
    return nc


def kernel(x: np.ndarray, kernel: np.ndarray) -> np.ndarray:
    global LAST_RESULTS
    if "nc" not in _CACHE:
        _CACHE["nc"] = _build()
    nc = _CACHE["nc"]

    x = np.ascontiguousarray(x, dtype=np.float32)
    kw_arr = np.ascontiguousarray(kernel, dtype=np.float32)

    xp = np.pad(x, ((0, 0), (1, 1), (1, 1)))          # [C, H+2, W+2]
    xp = xp.reshape(CB, 128, H + 2, W + 2)
    # w_t[b, ob, p, k, oc] = kernel[ob*128+oc, b*128+p, kh, kw] — each (b, ob)
    # quarter is contiguous per partition for a clean DMA line.
    w_t = np.ascontiguousarray(
        kw_arr.transpose(1, 2, 3, 0)
        .reshape(CB, 128, KH * KW, OB, 128)
        .transpose(0, 3, 1, 2, 4)
    )

    in_maps = []
    for i in range(N_CORES):
        xs_i = np.ascontiguousarray(xp[:, :, i * HS : i * HS + HS + 2, :])
        in_maps.append({"xs": xs_i, "w": w_t})

    results = run_bass_kernel_spmd(
        nc, in_maps, core_ids=list(range(N_CORES)), trace=TRACE
    )
    LAST_RESULTS = results

    parts = [r["out"].reshape(O, HS, W) for r in results.results]
    return np.concatenate(parts, axis=1)
